# revision 29
# baseline (speedup 1.0000x reference)
"""Self-contained Trainium2 Bass kernel for nn_ComplementarityScoreHead.

out = (h_norm @ h_norm.T) * edge_mask, h = MLP(x), h_norm = h / ||h||_2(rows)

Strategy (8 NeuronCores, SPMD, symmetric-band formulation):
  - The correlation matrix is symmetric, so each 128-row chunk only needs the
    4224-wide column window starting at its own diagonal (rolled frame): an
    edge (r, c) with forward circular distance d = (c-r) mod N is read from
    r's chunk when d <= 4095, else from c's chunk (distance 8192-d <= 4096).
  - Each core m receives x rolled by -1024*m and sliced to the first 5120
    rows (= max window extent 896 + 4224); one identical program runs on all
    cores.
  - Per core: fused per-512-column tile: PE-transpose of x (f32r), MLP layer1
    (relu) and layer2 (fp32r matmuls), sum-of-squares over the feature
    (partition) axis via an all-ones matmul, sqrt+reciprocal, and
    normalization into bf16 hnT tiles.
  - Band slab per chunk mt: columns [128*mt, 128*mt+4224) as bf16 matmuls
    (1 cycle/row at any tile width) accumulated over K=256 in PSUM; PSUM is
    evicted to a bf16 SBUF tile round-robin on DVE/Act/Pool and streamed to
    DRAM as a [128, 4224] bf16 chunk. No edge mask on chip.
  - Host glue: roll+slice x per core; gather the edge values from the 8
    banded bf16 slabs and scatter into the dense float32 [N, N] zeros.
"""
import sys
import numpy as np

sys.path.insert(0, '/opt/trn_rl_repo')

import concourse.bass as bass  # noqa: E402
import concourse.mybir as mybir  # noqa: E402
from concourse import bacc  # noqa: E402
from concourse.tile import TileContext  # noqa: E402
from concourse.masks import make_identity  # noqa: E402
from concourse.bass_utils import run_bass_kernel_spmd  # noqa: E402

N = 8192
F = 128
H = 256
NCORES = 8
SLAB = N // NCORES
CHUNKS = SLAB // 128
W = 4224               # band window width per chunk
NROWS = 128 * (CHUNKS - 1) + W  # 5120 rolled rows needed per core
NT = NROWS // 512      # 10 MLP column tiles
COLT = [1024, 1024, 1024, 1024, 128]  # per-chunk column tiling of W


def _build_nc():
    f32 = mybir.dt.float32
    f32r = mybir.dt.float32r
    bf16 = mybir.dt.bfloat16
    AF = mybir.ActivationFunctionType

    nc = bacc.Bacc()
    x = nc.declare_dram_parameter("x", [F, NROWS], f32r, isOutput=False)
    W1 = nc.declare_dram_parameter("W1", [F, H], f32, isOutput=False)
    b1 = nc.declare_dram_parameter("b1", [128, 2], f32, isOutput=False)
    W2 = nc.declare_dram_parameter("W2", [128, 2, H], f32, isOutput=False)
    b2 = nc.declare_dram_parameter("b2", [128, 2], f32, isOutput=False)
    out = nc.declare_dram_parameter("out", [SLAB, W], bf16, isOutput=True)

    with TileContext(nc) as tc:
        with (
            tc.tile_pool(name="singles", bufs=1) as singles,
            tc.tile_pool(name="hn", bufs=2) as hn_pool,
            tc.tile_pool(name="mid", bufs=3) as mid,
            tc.tile_pool(name="ob", bufs=1) as ob_pool,
        ):
            # x arrives pre-transposed from the host: stream it in 1024-col
            # chunks on the SP queue so layer 1 can start immediately
            xT = singles.tile([128, NROWS], f32r)
            xoff = 0
            for cw in (512, 512, 1024, 1024, 1024, 1024):
                nc.sync.dma_start(out=xT[:, xoff:xoff + cw],
                                  in_=x[:, xoff:xoff + cw])
                xoff += cw
            # weights go on the gpsimd DMA queue so the x stream owns nc.sync
            w1f = singles.tile([128, H], f32)
            nc.gpsimd.dma_start(out=w1f[:], in_=W1[:])
            w1r = singles.tile([128, H], f32r)
            nc.vector.tensor_copy(w1r[:], w1f[:])
            w2f = singles.tile([128, 2, H], f32)
            nc.gpsimd.dma_start(out=w2f[:], in_=W2[:])
            w2r = singles.tile([128, 2, H], f32r)
            nc.vector.tensor_copy(w2r[:], w2f[:])
            b1s = singles.tile([128, 2], f32)
            nc.gpsimd.dma_start(out=b1s[:], in_=b1[:])
            b2s = singles.tile([128, 2], f32)
            nc.gpsimd.dma_start(out=b2s[:], in_=b2[:])

            # unnormalized bf16 h, transposed; normalization happens on the
            # host using the band diagonal (= ||h||^2 per node)
            hnb = [hn_pool.tile([128, NROWS], bf16, tag="hn", name=f"hnb{s}")
                   for s in range(2)]

            # MLP tiles and correlation band tiles are interleaved in one
            # emission stream: corr tile (mt, t) is emitted as soon as the
            # hnb columns it reads are produced, keeping the in-order PE
            # queue saturated while MLP evictions drain.
            psA_cm = tc.tile_pool(name="psA", bufs=4, space="PSUM")
            psB_cm = tc.tile_pool(name="psB", bufs=2, space="PSUM")
            psA = psA_cm.__enter__()
            psB = psB_cm.__enter__()
            if True:
                obs = [ob_pool.tile([128, W], bf16, tag=f"ob{mt}",
                                    name=f"ob{mt}")
                       for mt in range(CHUNKS)]
                ndone = [0] * CHUNKS
                ecnt = [0]

                def mlp_tile(nt):
                    sl = slice(nt * 512, (nt + 1) * 512)
                    r1s = mid.tile([128, 2, 512], f32r, tag="r1s")
                    for s in range(2):
                        ps = psA.tile([128, 512], f32, tag="ps")
                        nc.tensor.matmul(
                            ps[:], w1r[:, s * 128:(s + 1) * 128], xT[:, sl],
                            start=True, stop=True)
                        if s == 0:
                            nc.scalar.activation(r1s[:, s, :], ps[:], AF.Relu,
                                                 bias=b1s[:, s:s + 1])
                        else:
                            nc.vector.tensor_scalar(
                                r1s[:, s, :], ps[:], b1s[:, s:s + 1], 0.0,
                                mybir.AluOpType.add, mybir.AluOpType.max)
                    for s2 in range(2):
                        ps = psA.tile([128, 512], f32, tag="ps")
                        for k in range(2):
                            nc.tensor.matmul(
                                ps[:], w2r[:, k, s2 * 128:(s2 + 1) * 128],
                                r1s[:, k, :], start=(k == 0), stop=(k == 1))
                        if s2 == 0:
                            nc.scalar.activation(hnb[s2][:, sl], ps[:],
                                                 AF.Identity,
                                                 bias=b2s[:, s2:s2 + 1])
                        else:
                            nc.vector.tensor_scalar_add(hnb[s2][:, sl], ps[:],
                                                        b2s[:, s2:s2 + 1])

                def corr_tile(mt, pool=None):
                    base = 128 * mt
                    t = ndone[mt]
                    off = 1024 * t
                    wd = COLT[t]
                    ob = obs[mt]
                    ps = (pool or psB).tile([128, 1024], f32, tag="psb")
                    for sub in range(0, wd, 512):
                        w2 = min(512, wd - sub)
                        for k in range(2):
                            nc.tensor.matmul(
                                ps[:, sub:sub + w2],
                                hnb[k][:, base:base + 128],
                                hnb[k][:, base + off + sub:
                                        base + off + sub + w2],
                                start=(k == 0), stop=(k == 1))
                    if ecnt[0] % 5 in (0, 2):
                        nc.vector.tensor_copy(ob[:, off:off + wd], ps[:, :wd])
                    else:
                        nc.scalar.activation(ob[:, off:off + wd], ps[:, :wd],
                                             AF.Identity)
                    ecnt[0] += 1
                    ndone[mt] += 1
                    nc.sync.dma_start(
                        out=out[mt * 128:(mt + 1) * 128, off:off + wd],
                        in_=ob[:, off:off + wd])

                for nt in range(NT):
                    avail = 512 * nt
                    for mt in range(CHUNKS):
                        while (ndone[mt] < len(COLT)
                               and 128 * mt + 1024 * ndone[mt]
                               + COLT[ndone[mt]] <= avail):
                            corr_tile(mt)
                    mlp_tile(nt)
                psB_cm.__exit__(None, None, None)
                psA_cm.__exit__(None, None, None)
                with tc.tile_pool(name="psC", bufs=4,
                                  space="PSUM") as psC:
                    for mt in range(CHUNKS):
                        while ndone[mt] < len(COLT):
                            corr_tile(mt, psC)
    nc.compile()
    return nc


_NC_CACHE = {}


def _get_nc():
    if "nc" not in _NC_CACHE:
        _NC_CACHE["nc"] = _build_nc()
    return _NC_CACHE["nc"]


def _prep_in_maps(x, W1, b1, W2, b2):
    x = np.ascontiguousarray(np.asarray(x, dtype=np.float32))
    W1 = np.ascontiguousarray(np.asarray(W1, dtype=np.float32))
    W2h = np.ascontiguousarray(
        np.asarray(W2, dtype=np.float32).reshape(2, 128, H).transpose(1, 0, 2))
    b1h = np.ascontiguousarray(np.asarray(b1, dtype=np.float32).reshape(2, 128).T)
    b2h = np.ascontiguousarray(np.asarray(b2, dtype=np.float32).reshape(2, 128).T)
    in_maps = []
    for m in range(NCORES):
        xm = np.ascontiguousarray(np.roll(x, -SLAB * m, axis=0)[:NROWS].T)
        in_maps.append({"x": xm, "W1": W1, "b1": b1h, "W2": W2h, "b2": b2h})
    return in_maps


def _assemble(results, edge_index):
    # bands[m, p, j] = h[1024m+p] . h[(1024m + 128*(p//128) + j) mod N]
    bands = np.stack([np.asarray(results[m]["out"]).astype(np.float32)
                      for m in range(NCORES)])
    # band diagonal = ||h||^2 per node; normalize on the host
    p = np.arange(SLAB)
    ss = bands[:, p, p % 128].reshape(-1)
    rsqn = 1.0 / np.sqrt(np.maximum(ss, 1e-24))
    r = np.asarray(edge_index[0], dtype=np.int64)
    c = np.asarray(edge_index[1], dtype=np.int64)
    d = (c - r) % N
    jr = (d + (r % 128)) % N
    jc = (((r - c) % N) + (c % 128)) % N
    use_r = jr < W
    vr = bands[r // SLAB, r % SLAB, np.minimum(jr, W - 1)]
    vc = bands[c // SLAB, c % SLAB, np.minimum(jc, W - 1)]
    out = np.zeros((N, N), dtype=np.float32)
    out[r, c] = np.where(use_r, vr, vc) * rsqn[r] * rsqn[c]
    return out


def kernel(x, edge_index, W1, b1, W2, b2):
    nc = _get_nc()
    in_maps = _prep_in_maps(x, W1, b1, W2, b2)
    res = run_bass_kernel_spmd(nc, in_maps, list(range(NCORES)))
    return _assemble(res.results, edge_index)
